# revision 17
# baseline (speedup 1.0000x reference)
"""Trainium2 Bass kernel for a fused LSTM cell.

Reference math (B=8192, D=U=1024, all fp32):
    z = x @ Wx + h_tm1 @ Uh + b          # Wx=[W_i|W_f|W_c|W_o], Uh likewise
    i, f = sigmoid(z_i), sigmoid(z_f)
    c = f * c_tm1 + i * tanh(z_c)
    h = sigmoid(z_o) * tanh(c)
    returns (h, c)

Strategy:
  - Data-parallel over 8 NeuronCores: batch 8192 -> 1024 rows/core,
    weights replicated. No collectives.
  - Per core the GEMM is computed transposed: z^T [4096 units, 1024 batch].
    lhsT (stationary) = weight tiles [128k, 128n]; rhs (moving) =
    host-pretransposed [x|h]^T tiles [128k, 512 batch]. Units on PSUM
    partitions so the per-unit bias folds into the ScalarE activation.
  - GEMM operands are fp16 (PSUM accumulation fp32): same PE rate as
    fp32r (1 cycle/row) but half the HBM traffic, so the DMA stream
    stays ahead of the PE from the first tile. Measured end-to-end
    error vs the fp32 reference: ~2.3e-3 (quantization of x/h/W only).
  - A run of dummy matmuls on a zeroed tile right at kernel start keeps
    the PE busy through the initial DMA wait so the HAM clock gate is
    already at full rate (2.4 GHz) when the real matmuls begin.
  - Last unit-block: c-path epilogue issues after the i/f/c gates while
    the o-gate matmuls still run; o is computed in two half-width PSUM
    groups so activation + h-mul + store of the first half overlap the
    second half's matmuls.
"""

from contextlib import ExitStack

import numpy as np

import concourse.bass as bass
import concourse.tile as tile
from concourse import bacc, mybir
from concourse.bass_utils import run_bass_kernel_spmd

B, D, U = 8192, 1024, 1024
NCORES = 8
BS = B // NCORES  # per-core batch rows
F = 512           # moving-operand free size (PSUM bank limit)
WARMUP_MMS = 30   # dummy matmuls: keep the PE busy (no idle window, which
                  # would reset the HAM activity monitor) through the
                  # first-DMA completion latency so the clock gate is at
                  # 2.4GHz right as the real matmuls begin


def build_nc(bs=BS, d=D, u=U, f=F):
    """Build the per-core SPMD Bass program.

    DRAM parameter layouts (host prepares these, fp16 unless noted):
      xh   [KP, 128, 2, bs] : [x|h]^T; contraction k = (kp*2+a)*128 + p
      w    [JB, KP, 128, 2, 4, 128]:
           w[j,kp,p,a,g,n] = W_all[(kp*2+a)*128+p, (g*JB+j)*128+n]
      bias [128, NT] fp32   : bias[p, t] = b_all[t*128+p]
      ct   [JB, 128, bs] fp32 : c_tm1^T unit-blocks
      h_out/c_out [JB, 128, bs] fp32 : h^T / c^T unit-blocks
    """
    kdim = d + u
    KO = kdim // 128   # contraction blocks
    KP = KO // 2       # ko pairs (DMA granularity)
    JB = u // 128      # unit blocks per gate
    NT = 4 * u // 128  # total n-tiles (4 gates)
    BH = bs // f       # batch chunks of the moving operand

    f32 = mybir.dt.float32
    f16 = mybir.dt.float16
    SIG = mybir.ActivationFunctionType.Sigmoid
    TANH = mybir.ActivationFunctionType.Tanh

    nc = bacc.Bacc("TRN2", target_bir_lowering=False, debug=False)

    xh = nc.dram_tensor("xh", [KP, 128, 2, bs], f16, kind="ExternalInput").ap()
    w = nc.dram_tensor("w", [JB, KP, 128, 2, 4, 128], f16, kind="ExternalInput").ap()
    bia = nc.dram_tensor("bias", [128, NT], f32, kind="ExternalInput").ap()
    ct = nc.dram_tensor("ct", [JB, 128, bs], f32, kind="ExternalInput").ap()
    ho = nc.dram_tensor("h_out", [JB, 128, bs], f32, kind="ExternalOutput").ap()
    co = nc.dram_tensor("c_out", [JB, 128, bs], f32, kind="ExternalOutput").ap()

    with tile.TileContext(nc) as tc, ExitStack() as ctx:
        warm_pool = ctx.enter_context(tc.tile_pool(name="warm", bufs=1))
        xh_pool = ctx.enter_context(tc.tile_pool(name="xh", bufs=1))
        w_pool = ctx.enter_context(tc.tile_pool(name="w", bufs=2 * KP))
        bias_pool = ctx.enter_context(tc.tile_pool(name="bias", bufs=1))
        ct_pool = ctx.enter_context(tc.tile_pool(name="ct", bufs=2))
        gate_pool = ctx.enter_context(tc.tile_pool(name="gates", bufs=2))
        out_pool = ctx.enter_context(tc.tile_pool(name="outs", bufs=2))
        psum_pool = ctx.enter_context(tc.tile_pool(name="psum", bufs=8, space="PSUM"))

        # --- PE warmup: dummy matmuls on a zeroed tile, no DMA deps.
        warm_sb = warm_pool.tile([128, 128], f16, tag="warm")
        nc.gpsimd.memset(warm_sb[:], 0.0)
        warm_ps = psum_pool.tile([128, f], f32, tag="ps", name="warm_ps")
        for _ in range(WARMUP_MMS):
            nc.tensor.matmul(
                warm_ps[:, :128], lhsT=warm_sb[:], rhs=warm_sb[:],
                start=True, stop=True,
            )

        # Two HWDGE rings: weights + outputs on the sync ring, xh/ct/bias on
        # the scalar ring. Two outstanding streams hide per-transfer receipt
        # latency during the startup burst when all 8 cores hit HBM at once.
        def load_ct(j):
            t = ct_pool.tile([128, bs], f32, tag="ct")
            nc.scalar.dma_start(t[:], ct[j])
            return t

        def load_wk(j, kp, split=False):
            t = w_pool.tile([128, 2, 4, 128], f16, tag="wk", name=f"wk_{j}_{kp}")
            if split:
                nc.sync.dma_start(t[:, 0], w[j, kp][:, 0])
                nc.sync.dma_start(t[:, 1], w[j, kp][:, 1])
            else:
                nc.sync.dma_start(t[:], w[j, kp])
            return t

        # Startup: j=0's weights stream in consumption order on the sync
        # ring while xh spreads over the scalar HWDGE ring and the gpsimd
        # SWDGE ring. During the initial burst (all 8 cores fetching at
        # once) completions serialize per ring, so the first wave is kept
        # small (128KB/ring) and xh0's four (a, bh) quarters land across
        # two rings in the order the kp0 matmuls consume them.
        xh_sb = []
        wk_by_j = {0: []}
        for kp in range(KP):
            wk_by_j[0].append(load_wk(0, kp, split=(kp == 0)))
        bh1 = slice(f, bs)
        for kp in range(KP):
            t = xh_pool.tile([128, 2, bs], f16, tag=f"xh{kp}", name=f"xh{kp}")
            if kp == 0:
                nc.scalar.dma_start(t[:, 0, :f], xh[kp][:, 0, :f])
                nc.gpsimd.dma_start(t[:, 1, :f], xh[kp][:, 1, :f])
                nc.gpsimd.dma_start(t[:, 0, bh1], xh[kp][:, 0, bh1])
                nc.scalar.dma_start(t[:, 1, bh1], xh[kp][:, 1, bh1])
            elif kp % 2 == 0:
                nc.gpsimd.dma_start(t[:], xh[kp])
            else:
                nc.scalar.dma_start(t[:], xh[kp])
            xh_sb.append(t)
        bias_sb = bias_pool.tile([128, NT], f32, tag="bias")
        nc.scalar.dma_start(bias_sb[:], bia[:])
        ct_by_j = {0: load_ct(0)}  # not needed until j=0's epilogue

        def epilogue(j, bh, gt, ct_sb, h_out, c_out):
            bsl = slice(bh * f, (bh + 1) * f)
            t1 = gate_pool.tile([128, f], f32, tag="t1")
            nc.vector.tensor_mul(t1[:], gt[1][:], ct_sb[:, bsl])
            t2 = gate_pool.tile([128, f], f32, tag="t2")
            nc.vector.tensor_mul(t2[:], gt[0][:], gt[2][:])
            nc.vector.tensor_add(c_out[:, bsl], t1[:], t2[:])
            tct = gate_pool.tile([128, f], f32, tag="tct")
            nc.scalar.activation(tct[:], c_out[:, bsl], TANH)
            nc.vector.tensor_mul(h_out[:, bsl], gt[3][:], tct[:])
            if bh == BH - 1:
                # one store per unit-block (fewer DMAs, off critical path)
                nc.sync.dma_start(ho[j][:], h_out[:])
                nc.sync.dma_start(co[j][:], c_out[:])

        def act_gate(j, g, ps, width=f, name="gt"):
            gtile = gate_pool.tile([128, width], f32, tag=f"g{g}", name=name)
            idx = g * JB + j
            func = TANH if g == 2 else SIG
            nc.scalar.activation(
                gtile[:], ps[:], func, bias=bias_sb[:, idx : idx + 1]
            )
            return gtile

        def mm_group(j, g, bh, wk, csl=slice(0, f), name="ps"):
            """Accumulate one gate tile over the full contraction."""
            psb = psum_pool.tile([128, csl.stop - csl.start], f32, tag="ps", name=name)
            for kp in range(KP):
                for a in range(2):
                    nc.tensor.matmul(
                        psb[:],
                        lhsT=wk[kp][:, a, g, :],
                        rhs=xh_sb[kp][:, a, bh * f + csl.start : bh * f + csl.stop],
                        start=(kp == 0 and a == 0),
                        stop=(kp == KP - 1 and a == 1),
                    )
            return psb

        for j in range(JB):
            # prefetch next block's weights/ct one block ahead
            if j + 1 < JB and (j + 1) not in wk_by_j:
                wk_by_j[j + 1] = [load_wk(j + 1, kp) for kp in range(KP)]
            if j + 1 < JB and (j + 1) not in ct_by_j:
                ct_by_j[j + 1] = load_ct(j + 1)
            wk = wk_by_j.pop(j)
            ct_sb = ct_by_j.pop(j)
            h_out = out_pool.tile([128, bs], f32, tag="h")
            c_out = out_pool.tile([128, bs], f32, tag="c")
            if j == 0:
                # ko-major: all 8 (g, bh) groups accumulate together so the
                # PE chases the arriving xh/w DMA stream tile by tile.
                ps = [
                    [
                        psum_pool.tile(
                            [128, f], f32, tag="ps", name=f"ps_{g}_{bh}"
                        )
                        for bh in range(BH)
                    ]
                    for g in range(4)
                ]
                for kp in range(KP):
                    for bh in range(BH):
                        for a in range(2):
                            for g in range(4):
                                nc.tensor.matmul(
                                    ps[g][bh][:],
                                    lhsT=wk[kp][:, a, g, :],
                                    rhs=xh_sb[kp][:, a, bh * f : (bh + 1) * f],
                                    start=(kp == 0 and a == 0),
                                    stop=(kp == KP - 1 and a == 1),
                                )
                for bh in range(BH):
                    gt = [act_gate(j, g, ps[g][bh]) for g in range(4)]
                    epilogue(j, bh, gt, ct_sb, h_out, c_out)
            elif j < JB - 1:
                for bh in range(BH):
                    gt = []
                    for g in range(4):
                        psb = mm_group(j, g, bh, wk)
                        gt.append(act_gate(j, g, psb))
                    epilogue(j, bh, gt, ct_sb, h_out, c_out)
            else:
                # Last block: bh=0 normal; for the final bh compute the
                # c-path as soon as i/f/c are done (overlapping o's matmuls)
                # and split o into two half-width groups so the tail after
                # the very last matmul is one small act+mul+store.
                for bh in range(BH - 1):
                    gt = []
                    for g in range(4):
                        psb = mm_group(j, g, bh, wk)
                        gt.append(act_gate(j, g, psb))
                    epilogue(j, bh, gt, ct_sb, h_out, c_out)
                bh = BH - 1
                bsl = slice(bh * f, (bh + 1) * f)
                gt = []
                for g in range(3):
                    psb = mm_group(j, g, bh, wk)
                    gt.append(act_gate(j, g, psb))
                # c-path (runs on Vector/Scalar while o's matmuls proceed)
                t1 = gate_pool.tile([128, f], f32, tag="t1")
                nc.vector.tensor_mul(t1[:], gt[1][:], ct_sb[:, bsl])
                t2 = gate_pool.tile([128, f], f32, tag="t2")
                nc.vector.tensor_mul(t2[:], gt[0][:], gt[2][:])
                nc.vector.tensor_add(c_out[:, bsl], t1[:], t2[:])
                tct = gate_pool.tile([128, f], f32, tag="tct")
                nc.scalar.activation(tct[:], c_out[:, bsl], TANH)
                nc.sync.dma_start(co[j][:], c_out[:])
                # o-gate in two PSUM groups; the wide group's h-mul/store is
                # chunked so the chain after the very last matmul is short.
                q = f // 4
                psb = mm_group(j, 3, bh, wk, csl=slice(0, q), name="ps_o0")
                og = act_gate(j, 3, psb, width=q, name="og0")
                nc.vector.tensor_mul(h_out[:, bh * f : bh * f + q], og[:], tct[:, :q])
                nc.sync.dma_start(
                    ho[j][:, : bh * f + q], h_out[:, : bh * f + q]
                )
                psb = mm_group(j, 3, bh, wk, csl=slice(q, f), name="ps_o1")
                og = act_gate(j, 3, psb, width=f - q, name="og1")
                for ci in range(2):
                    csl = slice(q + ci * (f - q) // 2, q + (ci + 1) * (f - q) // 2)
                    hsl = slice(bh * f + csl.start, bh * f + csl.stop)
                    nc.vector.tensor_mul(
                        h_out[:, hsl], og[:, csl.start - q : csl.stop - q], tct[:, csl]
                    )
                    nc.sync.dma_start(ho[j][:, hsl], h_out[:, hsl])

    nc.compile()
    return nc


def pack_shared(inputs):
    """Weight + bias device arrays (replicated on every core)."""
    d, u = inputs["W_i"].shape[0], inputs["W_i"].shape[1]
    kdim = d + u
    KP = kdim // 256
    JB = u // 128
    NT = 4 * u // 128
    Wx = np.concatenate(
        [inputs["W_i"], inputs["W_f"], inputs["W_c"], inputs["W_o"]], axis=1
    )
    Uh = np.concatenate(
        [inputs["U_i"], inputs["U_f"], inputs["U_c"], inputs["U_o"]], axis=1
    )
    W_all = np.concatenate([Wx, Uh], axis=0)  # [kdim, 4u]
    # w[j,kp,p,a,g,n] = W_all[(kp*2+a)*128+p, (g*JB+j)*128+n]
    w_dev = np.ascontiguousarray(
        W_all.reshape(KP, 2, 128, 4, JB, 128).transpose(4, 0, 2, 1, 3, 5)
    ).astype(np.float16)
    b_all = np.concatenate(
        [inputs["b_i"], inputs["b_f"], inputs["b_c"], inputs["b_o"]]
    )  # [4u]
    b_dev = np.ascontiguousarray(b_all.reshape(NT, 128).T).astype(np.float32)
    return w_dev, b_dev


def pack_core(x_i, h_i, c_i):
    """Per-core shard arrays."""
    bs = x_i.shape[0]
    d, u = x_i.shape[1], h_i.shape[1]
    KP = (d + u) // 256
    JB = u // 128
    xh_t = np.concatenate([x_i, h_i], axis=1).T  # [kdim, bs]
    xh_dev = np.ascontiguousarray(
        xh_t.reshape(KP, 2, 128, bs).transpose(0, 2, 1, 3)
    ).astype(np.float16)
    ct_dev = np.ascontiguousarray(c_i.T.reshape(JB, 128, bs)).astype(np.float32)
    return xh_dev, ct_dev


_NC_CACHE = {}


def _get_nc():
    key = (BS, D, U)
    if key not in _NC_CACHE:
        _NC_CACHE[key] = build_nc()
    return _NC_CACHE[key]


def build_in_maps(inputs):
    x = np.asarray(inputs["inputs"], np.float32)
    h = np.asarray(inputs["h_tm1"], np.float32)
    c = np.asarray(inputs["c_tm1"], np.float32)
    w_dev, b_dev = pack_shared(inputs)
    in_maps = []
    for i in range(NCORES):
        sl = slice(i * BS, (i + 1) * BS)
        xh_dev, ct_dev = pack_core(x[sl], h[sl], c[sl])
        in_maps.append({"xh": xh_dev, "w": w_dev, "bias": b_dev, "ct": ct_dev})
    return in_maps


def _run(inputs, trace=False):
    in_maps = build_in_maps(inputs)
    nc = _get_nc()
    res = run_bass_kernel_spmd(nc, in_maps, list(range(NCORES)), trace=trace)
    u = U
    h_full = np.empty((B, u), np.float32)
    c_full = np.empty((B, u), np.float32)
    for i in range(NCORES):
        sl = slice(i * BS, (i + 1) * BS)
        h_full[sl] = res.results[i]["h_out"].reshape(u, BS).T
        c_full[sl] = res.results[i]["c_out"].reshape(u, BS).T
    return (h_full, c_full), res


def kernel(**inputs):
    out, _ = _run(inputs, trace=False)
    return out
